# revision 21
# baseline (speedup 1.0000x reference)
"""Trainium2 Bass kernel for nn_CRF_Layer (CRF loss gradients).

Computes gradients = concat(mean_dw [26*128], mean_dT [26*26]) for a batch
of 512 words, data-parallel over 8 NeuronCores (64 words/core).

Algorithm (per core, Wc=64 words, m=256, K=26, D=128, P=Wc*m=16384):
  - scores^T = W @ x^T via PE (fp16 operands, f32 accum), ES = exp(scores^T)
  - forward/backward CRF recursions run in *exp space*: the recursion
    ea_{i+1} = (ea_i * es_i) @ expTs is linear, with expTs = exp(T - 3.9)
    rescaled to keep magnitudes bounded. Sequence is split into S=8
    segments recursed in parallel (stacked in the matmul free dim); each
    segment starts from ones with B=4 burn-in steps (the recursion is
    exponentially contracting, so boundary values converge to f32 noise).
    fwd and bwd are stacked on partitions (fwd rows 0:26, bwd rows 32:58)
    sharing one DVE mul + one PE matmul per step.
  - u_i = ea_i*es_i (fwd mul output), v_i = eb_i*es_i (bwd mul output)
    stored fp16; EB_i = expTs @ v_{i+1} recovered by a bulk matmul. Then
    p1 numerator q' = u*EB, Z = sum_k q', and the gradient matmuls
    dw -= qhat^T x, p2sum = uhat^T v+, counts = oh^T oh+ all run as one
    fused PE pass with lhsT=[G|uhat|oh], rhs=[x|v+|oh+].
  - per-position normalization makes all per-segment scales cancel.

Position convention for "bi-major" tensors: position p <-> SBUF
(partition p & 127, column p >> 7), matching the row order produced by the
DMA xbar transpose of k-major [*, 16384] tensors (c-major rows).
"""

import os
import numpy as np

import concourse.bass as bass
import concourse.mybir as mybir
import concourse.tile as tile
from concourse import bacc
from concourse.bass_utils import run_bass_kernel_spmd

K = 26
D = 128
M = 256          # word length
WC = 64          # words per core
P = WC * M       # positions per core = 16384
NCORES = 8
S = 8            # recursion segments
B = 4            # burn-in steps
L = M // S       # segment length = 32
CSCALE = 3.9     # exp-space rescale folded into expTs
NCH = P // 128   # 128 chunks of 128 positions

F16 = mybir.dt.float16
F32 = mybir.dt.float32
I32 = mybir.dt.int32

# fused grad-mm column layout
#   lhsT: [G(0:26) | uhat(26:52) | oh(52:78)]           width 78
#   rhs:  [x(0:128) | vplus(128:154) | ohp(154:180)]    width 180
LW = 78
RW = 180


def _ap(t, offset, dims):
    return bass.AP(tensor=t.tensor, offset=t.offset + offset,
                   ap=[list(d) for d in dims])


def build_program(tc, outs, ins):
    nc = tc.nc
    x_dram = ins["x"]          # [P, D] fp16 (w-major positions)
    lab_dram = ins["labels"]   # [P] int32
    labn_dram = ins["labels_next"]  # [P] int32, labels[p+1] w/ 99 at word ends
    w_dram = ins["W"]          # [K, D] f32
    t_dram = ins["T"]          # [K, K] f32
    dw_out = outs["dw"]        # [K, D] f32
    dt_out = outs["dT"]        # [K, K] f32

    exp = mybir.ActivationFunctionType.Exp

    import contextlib
    with contextlib.ExitStack() as ctx:
        persist = ctx.enter_context(tc.tile_pool(name="persist", bufs=1))
        psum_small = ctx.enter_context(
            tc.tile_pool(name="ps_small", bufs=1, space="PSUM"))

        # ---------------- phase 0: constants ----------------
        # W^T fp16 [128, 26]
        wsb = persist.tile([K, D], F32)
        nc.sync.dma_start(out=wsb, in_=w_dram)
        wt32 = persist.tile([D, K], F32)
        nc.sync.dma_start(out=wt32, in_=w_dram.rearrange("k d -> d k"))
        wt = persist.tile([D, K], F16)
        nc.vector.tensor_copy(wt, wt32)

        tsb = persist.tile([K, K], F32)
        nc.sync.dma_start(out=tsb, in_=t_dram)
        tt32 = persist.tile([K, K], F32)
        nc.sync.dma_start(out=tt32, in_=t_dram.rearrange("a b -> b a"))

        # bias tiles for activation calls (bias must be an AP for Exp)
        nbias = persist.tile([64, 1], F32)
        nc.vector.memset(nbias, -CSCALE)
        zbias = persist.tile([64, 1], F32)
        nc.vector.memset(zbias, 0.0)

        # expTs f32 (for final dT combine)
        expts32 = persist.tile([K, K], F32)
        nc.scalar.activation(expts32, tsb, exp, bias=nbias[0:K])

        # block-diag lhsT LT [64, 64] fp16: [0:26,0:26]=expTs, [32:58,32:58]=expTs^T
        lt = persist.tile([64, 64], F16)
        nc.vector.memset(lt, 0.0)
        nc.scalar.activation(lt[0:K, 0:K], tsb, exp, bias=nbias[0:K])
        nc.scalar.activation(lt[32:32 + K, 32:32 + K], tt32, exp, bias=nbias[0:K])

        # iota [128, 26] int32 (same 0..25 on every partition)
        iota_t = persist.tile([128, K], I32)
        nc.gpsimd.iota(iota_t, pattern=[[1, K]], base=0, channel_multiplier=0)

        # labels (p-major bi-layout): lab0[p], lab1[p] = labels[p+1]
        lab0 = persist.tile([128, NCH], I32)
        lab1 = persist.tile([128, NCH], I32)
        nc.sync.dma_start(out=lab0, in_=lab_dram.rearrange("(c p) -> p c", p=128))
        nc.sync.dma_start(out=lab1, in_=labn_dram.rearrange("(c p) -> p c", p=128))

        # persistent big tiles
        rhs_t = persist.tile([128, NCH, RW], F16)      # grad-mm rhs
        uvt = persist.tile([64, P], F16)               # U rows 0:26 (nat), V rows 32:58 (rev)
        z_t = persist.tile([128, NCH], F32)
        rz_t = persist.tile([128, NCH], F32)
        rzn_t = persist.tile([128, NCH], F32)

        # x natural load into rhs cols 0:128 (p-major)
        nc.sync.dma_start(
            out=rhs_t[:, :, 0:D],
            in_=x_dram.rearrange("(c p) d -> p c d", p=128))

        # ---------------- phase 1: scores + ES ----------------
        with tc.tile_pool(name="esp", bufs=1) as esp:
            es = esp.tile([64, P], F32)                # rows 0:26 fwd, 32:58 bwd(rev)
            # zero the whole tile first: rows 26:32 / 58:64 are dead lanes that
            # must stay finite for the chain mul (partition slices must start at
            # a multiple of 32, so they can't be memset individually)
            nc.gpsimd.memset(es, 0.0)

            with tc.tile_pool(name="xtp", bufs=1) as xtp, \
                 tc.tile_pool(name="ph1ps", bufs=4, space="PSUM") as ph1ps:
                xt = xtp.tile([D, P], F16)             # x^T via DMA xbar
                nc.sync.dma_start_transpose(out=xt, in_=x_dram)

                for n in range(P // 512):
                    ps = ph1ps.tile([K, 512], F32)
                    nc.tensor.matmul(ps, lhsT=wt, rhs=xt[:, n * 512:(n + 1) * 512],
                                     start=True, stop=True)
                    nc.scalar.activation(es[0:K, n * 512:(n + 1) * 512], ps, exp, bias=zbias[0:K])

            # reversed copy for bwd rows: es[32+k, 256w + i] = es[k, 256w + 255-i]
            # split across DVE / ACT / POOL to shorten the serial gap
            src = es[0:K, :].rearrange("k (w i) -> k w i", w=WC)
            dst = es[32:32 + K, :].rearrange("k (w i) -> k w i", w=WC)
            splits = [(0, 21, nc.vector.tensor_copy),
                      (21, 42, nc.gpsimd.tensor_copy)]
            for w0, w1, op in splits:
                op(dst[:, w0:w1, :], src[:, w0:w1, ::-1])
            nc.scalar.activation(dst[:, 42:WC, :], src[:, 42:WC, ::-1],
                                 mybir.ActivationFunctionType.Copy)

            # ---------------- phase 2: stacked recursion ----------------
            with tc.tile_pool(name="chain", bufs=1) as chp, \
                 tc.tile_pool(name="chps", bufs=2, space="PSUM") as chps:
                scratch = chp.tile([64, (S - 1) * WC], F16)
                st = [chps.tile([64, S * WC], F32, name=f'state{i}', tag=f'state{i}') for i in range(2)]
                for t_ in st:
                    nc.vector.memset(t_, 0.0)
                    nc.vector.memset(t_[0:K, :], 1.0)
                    nc.vector.memset(t_[32:32 + K, :], 1.0)

                es_v = es.rearrange("p (w s l) -> p s w l", w=WC, s=S)
                uv_v = uvt.rearrange("p (w s l) -> p s w l", w=WC, s=S)
                sc_v = scratch.rearrange("p (s w) -> p s w", s=S - 1)

                for j in range(B + L):
                    cur, nxt = st[j % 2], st[(j + 1) % 2]
                    cur_v = cur.rearrange("p (s w) -> p s w", s=S)
                    if j < B:
                        # burn-in: segments 1..S-1 read ES col (s*L - B + j)
                        # = seg index (s-1) at l = L - B + j in the (s, l) view
                        mul_out = sc_v[:, :, :]
                        nc.vector.tensor_mul(
                            mul_out, cur_v[:, 1:S, :],
                            es_v[:, 0:S - 1, :, L - B + j])
                        nc.tensor.matmul(
                            nxt.rearrange("p (s w) -> p s w", s=S)[:, 1:S, :],
                            lhsT=lt, rhs=mul_out, start=True, stop=True)
                    else:
                        mul_out = uv_v[:, :, :, j - B]
                        nc.vector.tensor_mul(mul_out, cur_v[:, :, :],
                                             es_v[:, :, :, j - B])
                        if j < B + L - 1:
                            nc.tensor.matmul(nxt, lhsT=lt,
                                             rhs=mul_out, start=True, stop=True)

        # ---------------- phase 3a: EB + transposes ----------------
        with tc.tile_pool(name="ph3", bufs=1) as ph3, \
             tc.tile_pool(name="ph3ps", bufs=4, space="PSUM") as ph3ps:
            lhs_t = ph3.tile([128, NCH, LW], F16)      # grad-mm lhsT
            ut_t = ph3.tile([128, NCH, 32], F16)       # U^T bi-major
            ebt_t = ph3.tile([128, NCH, 32], F16)      # EB^T bi-major
            vpt_t = ph3.tile([128, NCH, 32], F16)      # (v+)^T bi-major
            qp_t = ph3.tile([128, NCH, K], F16)        # q'
            ph3a_ctx = tc.tile_pool(name="ph3a", bufs=1)
            ph3a = ph3a_ctx.__enter__()
            ebk = ph3a.tile([32, P], F16)
            vpk = ph3a.tile([32, P], F16)              # v+ k-major (nat cols)
            uv_pitch = uvt.ap[0][0]
            for n in range(P // 512):
                # rhs: v_{p+1} read from rev-stored V: per word w,
                # position 256w + i (i<=254) -> rev col 256w + 254 - i
                ps = ph3ps.tile([32, 512], F32)
                for wq in range(2):
                    rhs = _ap(uvt, 32 * uv_pitch + 512 * n + 256 * wq + 254,
                              [[uv_pitch, 32], [-1, 255]])
                    nc.tensor.matmul(ps[:, 256 * wq:256 * wq + 255],
                                     lhsT=lt[32:64, 32:64], rhs=rhs,
                                     start=True, stop=True)
                ek_v = ebk[:, n * 512:(n + 1) * 512].rearrange(
                    "p (w i) -> p w i", w=2)[:, :, 0:255]
                ps_v = ps.rearrange("p (w i) -> p w i", w=2)[:, :, 0:255]
                nc.scalar.activation(ek_v, ps_v,
                                     mybir.ActivationFunctionType.Copy)
            # EB at i=255 := 1.0  (true beta=0 there)
            ei = ebk.rearrange("p (w i) -> p w i", w=WC)
            nc.vector.memset(ei[:, :, 255], 1.0)

            # v+ k-major: vpk[:, 256w + i] = v_{p+1} = uvt[32:64, 256w + 254 - i]
            # (i <= 254; i = 255 zeroed -- kills i=255 terms in the p2 matmul)
            up = uvt.ap[0][0]
            vpk_v = vpk.rearrange("p (w i) -> p w i", w=WC)
            for w0, w1, op in ((0, 21, nc.vector.tensor_copy),
                               (21, 42, nc.gpsimd.tensor_copy)):
                op(vpk_v[:, w0:w1, 0:255],
                   _ap(uvt, 32 * up + 254 + 256 * w0,
                       [[up, 32], [256, w1 - w0], [-1, 255]]))
            nc.scalar.activation(
                vpk_v[:, 42:WC, 0:255],
                _ap(uvt, 32 * up + 254 + 256 * 42, [[up, 32], [256, WC - 42], [-1, 255]]),
                mybir.ActivationFunctionType.Copy)
            nc.vector.memset(vpk_v[:, :, 255], 0.0)

            # xbar transposes -> bi-major (p-major rows)
            nc.sync.dma_start_transpose(out=ut_t, in_=uvt[0:32, :])
            nc.sync.dma_start_transpose(out=vpt_t, in_=vpk)
            nc.sync.dma_start_transpose(out=ebt_t, in_=ebk)

            # v+ into rhs cols 128:154
            nc.vector.tensor_copy(rhs_t[:, :, D:D + K], vpt_t[:, :, 0:K])
            ph3a_ctx.__exit__(None, None, None)

            # ---------------- phase 3b: bi-major elementwise ----------------
            nc.vector.tensor_mul(qp_t, ut_t[:, :, 0:K], ebt_t[:, :, 0:K])
            nc.vector.tensor_reduce(z_t, qp_t, axis=mybir.AxisListType.X,
                                    op=mybir.AluOpType.add)
            nc.vector.reciprocal(rz_t, z_t)
            nc.vector.tensor_scalar_mul(rzn_t, rz_t, -1.0)

            rz_b = _ap(rz_t, 0, [[rz_t.ap[0][0], 128], [1, NCH], [0, K]])
            rzn_b = _ap(rzn_t, 0, [[rzn_t.ap[0][0], 128], [1, NCH], [0, K]])

            qn = ph3.tile([128, NCH, K], F16)          # -qhat
            nc.vector.tensor_mul(qn, qp_t, rzn_b)
            # uhat -> lhsT cols 26:52
            nc.vector.tensor_mul(lhs_t[:, :, K:2 * K], ut_t[:, :, 0:K], rz_b)

            # oh -> lhsT cols 52:78 ; ohp -> rhs cols 154:180
            lab0_b = _ap(lab0, 0, [[lab0.ap[0][0], 128], [1, NCH], [0, K]])
            lab1_b = _ap(lab1, 0, [[lab1.ap[0][0], 128], [1, NCH], [0, K]])
            iota_b = _ap(iota_t, 0, [[iota_t.ap[0][0], 128], [0, NCH], [1, K]])
            nc.vector.tensor_tensor(lhs_t[:, :, 2 * K:3 * K], lab0_b, iota_b,
                                    op=mybir.AluOpType.is_equal)
            nc.vector.tensor_tensor(rhs_t[:, :, D + K:D + 2 * K], lab1_b, iota_b,
                                    op=mybir.AluOpType.is_equal)

            # G = oh + (-qhat) -> lhsT cols 0:26
            nc.vector.tensor_add(lhs_t[:, :, 0:K], lhs_t[:, :, 2 * K:3 * K], qn)

            # ---------------- phase 3c: gradient matmuls ----------------
            gps1 = ph3ps.tile([K, D], F32, bufs=1)      # G^T x
            gps2 = ph3ps.tile([K, K], F32, bufs=1)      # uhat^T v+
            gps3 = ph3ps.tile([K, K], F32, bufs=1)      # oh^T oh+
            for c in range(NCH):
                st_, sp_ = (c == 0), (c == NCH - 1)
                nc.tensor.matmul(gps1, lhsT=lhs_t[:, c, 0:K],
                                 rhs=rhs_t[:, c, 0:D], start=st_, stop=sp_)
                nc.tensor.matmul(gps2, lhsT=lhs_t[:, c, K:2 * K],
                                 rhs=rhs_t[:, c, D:D + K], start=st_, stop=sp_)
                nc.tensor.matmul(gps3, lhsT=lhs_t[:, c, 2 * K:3 * K],
                                 rhs=rhs_t[:, c, D + K:D + 2 * K],
                                 start=st_, stop=sp_)

            # ---------------- finals ----------------
            dw_sb = ph3.tile([K, D], F32)
            nc.vector.tensor_copy(dw_sb, gps1)
            nc.sync.dma_start(out=dw_out, in_=dw_sb)

            t1 = ph3.tile([K, K], F32)
            nc.vector.tensor_mul(t1, expts32, gps2)
            dt_sb = ph3.tile([K, K], F32)
            nc.vector.tensor_sub(dt_sb, gps3, t1)
            nc.sync.dma_start(out=dt_out, in_=dt_sb)


_CACHE = {}


def _build_nc():
    nc = bacc.Bacc("TRN2", target_bir_lowering=False, debug=False,
                   num_devices=NCORES)
    ins = {
        "x": nc.dram_tensor("x", [P, D], F16, kind="ExternalInput").ap(),
        "labels": nc.dram_tensor("labels", [P], I32, kind="ExternalInput").ap(),
        "labels_next": nc.dram_tensor("labels_next", [P], I32,
                                      kind="ExternalInput").ap(),
        "W": nc.dram_tensor("W", [K, D], F32, kind="ExternalInput").ap(),
        "T": nc.dram_tensor("T", [K, K], F32, kind="ExternalInput").ap(),
    }
    outs = {
        "dw": nc.dram_tensor("dw", [K, D], F32, kind="ExternalOutput").ap(),
        "dT": nc.dram_tensor("dT", [K, K], F32, kind="ExternalOutput").ap(),
    }
    with tile.TileContext(nc) as tc:
        build_program(tc, outs, ins)
    nc.compile()
    return nc


def kernel(data, labels, W, T):
    data = np.asarray(data)
    labels = np.asarray(labels)
    W = np.asarray(W, dtype=np.float32)
    T = np.asarray(T, dtype=np.float32)
    Bt = data.shape[0]
    wc = Bt // NCORES

    if "nc" not in _CACHE:
        _CACHE["nc"] = _build_nc()
    nc = _CACHE["nc"]

    lab_i32 = labels.reshape(Bt, M).astype(np.int32)
    lab_next = np.full((Bt, M), 99, dtype=np.int32)
    lab_next[:, :-1] = lab_i32[:, 1:]

    in_maps = []
    for c in range(NCORES):
        xc = np.ascontiguousarray(
            data[c * wc:(c + 1) * wc].reshape(P, D).astype(np.float16))
        in_maps.append({
            "x": xc,
            "labels": np.ascontiguousarray(lab_i32[c * wc:(c + 1) * wc].reshape(-1)),
            "labels_next": np.ascontiguousarray(
                lab_next[c * wc:(c + 1) * wc].reshape(-1)),
            "W": W,
            "T": T,
        })

    # the slim axon client here has no NTFF hook; the trace path would crash
    os.environ["BASS_NEVER_TRACE"] = "1"
    res = run_bass_kernel_spmd(nc, in_maps, core_ids=list(range(NCORES)))
    _CACHE["last_results"] = res
    dw = np.zeros((K, D), np.float64)
    dT = np.zeros((K, K), np.float64)
    for r in res.results:
        dw += r["dw"].astype(np.float64)
        dT += r["dT"].astype(np.float64)
    out = np.concatenate([(dw / Bt).reshape(-1), (dT / Bt).reshape(-1)])
    return out.astype(np.float32)


if __name__ == "__main__":
    import reference
    ins = reference.setup_inputs()
    out = kernel(**{k: np.asarray(v) for k, v in ins.items()})
    print(out.shape, out.dtype)
